# revision 52
# baseline (speedup 1.0000x reference)
"""Trainium2 Bass kernel for nn_MultiHeadAttentionBlock_49967649521921.

Reference computation (per batch b, x viewed as [C=512, N=1024]):
    q = Wq @ x ; k = Wk @ x ; v = Wv @ x          (1x1 convs, biases are zeros)
    per head h (8 heads, hd=64):
      scores[d,e] = sum_n q_h[d,n] k_h[e,n] / 8
      attn = softmax(scores, axis=e)
      out_h[d,n]  = sum_e attn[d,e] v_h[e,n]
    y[c',s'] = out[h, d, n] with c' = h*64 + n//16, s' = (n%16)*64 + d
    final = Wo @ y    -> reshape [512, 32, 32]

Sharding: data-parallel over batch. 16 batches / 8 cores = 2 per core.
No collectives; host scatters inputs and gathers outputs.

Device-side layouts (per core):
  x_sb  [128, 4, 1024]  channel-chunk-major view of x_b [C, N] (fp32/f32r)
  qt/kt [128, 8, 512]   bf16, spatial on partitions (q^T / k^T)
  v_sb  [128, 4, 1024]  bf16, [O, N] layout
  scoresT per head-PAIR in one [128,4,128] PSUM tile: matmul of the
  128-wide qt/kt window against itself gives both heads' scoresT in the
  two diagonal 64x64 blocks (bf16 runs 1 cycle/row at any width, so the
  discarded off-diagonal half is cheap). Softmax over partitions via a
  single batched ones-matmul column sum + reciprocal + one broadcast
  matmul. attn (normalized, transposed) lands in a BLOCK-DIAGONAL bf16
  tile atblk [128(e-pair), oc, 128(d-pair)] so attn@v for a head pair is
  ONE [128x128x128] bf16 matmul per 128-wide n-chunk: lhsT = v pair
  chunk (stationary), rhs = atblk -> po [128 n, 128 (d0|d1)].
  The reference's transpose(2,3).reshape scramble is realized as a DRAM
  bounce on async DMA rings (SBUF->SBUF static DMAs execute as
  engine-blocking DIRECT2D; DRAM DMAs do not): po halves are copied to
  ot tiles (bf16), forwarded per (oc, half, h') into scr[oc, h', n, d],
  and read back per oc as y_sb[:, oc, :] via a 3-dim rearrange.

Matmul dtypes: q/k/v/o projections run as float32r (full-rate PE at
moving>=256); the attention core runs bf16 (full rate at any width).
"""

import os
import sys

import numpy as np

for _p in ("/opt/trn_rl_repo",):
    if _p not in sys.path and os.path.isdir(_p):
        sys.path.insert(0, _p)

from contextlib import ExitStack

import concourse.bass as bass
import concourse.tile as tile
from concourse import bacc
from concourse import mybir
from concourse.bass_utils import run_bass_kernel_spmd

F32 = mybir.dt.float32
F32R = mybir.dt.float32r
BF16 = mybir.dt.bfloat16
AF = mybir.ActivationFunctionType

N_CORES = 8
B_PER_CORE = 2
C = 512
N = 1024
NH = 8
HD = 64

PROJ_DT = F32R   # q/k/v/o projections (moving free dim 512 -> full rate)


def _mm_cast(ap, dt):
    if ap.dtype == dt:
        return ap
    return ap.bitcast(dt)


def _split_excess_dma_waits(nc):
    """walrus' static-DMA (PSEUDO_DMA_DIRECT2D) encoding accepts a single
    sync-wait; Bacc's generate_event_semaphores only splits waits on compute
    instructions. Move excess DMA waits onto preceding EventSemaphore
    carriers (2 waits each) on the same engine queue."""
    for f in nc.m.functions:
        for blk in f.blocks:
            changed = False
            new_insts = []
            for inst in blk.instructions:
                si = inst.sync_info
                waits = list(si.on_wait) if si is not None and si.on_wait else []
                if inst.opcode in ("DMACopy", "DmaTransposeAnt") and len(waits) > 1:
                    keep, excess = waits[:1], waits[1:]
                    k = 0
                    while excess:
                        chunk, excess = excess[:2], excess[2:]
                        ev = mybir.InstEventSemaphore(
                            name=f"{inst.name}-evw{k}",
                            opcode="EventSemaphore",
                            engine=inst.engine,
                            sync_info=mybir.SyncInfo(on_wait=chunk, on_update=[]),
                        )
                        new_insts.append(ev)
                        k += 1
                    inst.sync_info = mybir.SyncInfo(
                        on_wait=keep, on_update=list(si.on_update or [])
                    )
                    changed = True
                new_insts.append(inst)
            if changed:
                blk.instructions = new_insts


def _serialize_transposes(nc):
    """DmaTransposeAnt fans out over all 16 DMA engines; two in-flight
    transposes interleave destructively. Chain them: each waits for the
    previous one's completion semaphore (cumulative +16 accounting, same
    program-order bookkeeping bacc uses for its own thresholds)."""
    from collections import defaultdict

    cum = defaultdict(int)
    prev = None
    for f in nc.m.functions:
        for blk in f.blocks:
            changed = False
            new_insts = []
            for inst in blk.instructions:
                si = inst.sync_info
                if si and si.on_update:
                    for u in si.on_update:
                        if u.ant_name and u.ant_name.startswith("DMA"):
                            cum[u.ant_name] += u.update_value
                if inst.opcode == "DmaTransposeAnt":
                    if prev is not None:
                        pname, pid, pcum = prev
                        ev = mybir.InstEventSemaphore(
                            name=f"{inst.name}-txchain",
                            opcode="EventSemaphore",
                            engine=inst.engine,
                            sync_info=mybir.SyncInfo(
                                on_wait=[
                                    mybir.SyncWait(
                                        sync_type="semaphore",
                                        id=pid,
                                        ant_name=pname,
                                        wait_mode="sem-ge-imm",
                                        wait_value=pcum,
                                    )
                                ],
                                on_update=[],
                            ),
                        )
                        new_insts.append(ev)
                        changed = True
                    u0 = (si.on_update or [None])[0]
                    assert u0 is not None, f"transpose {inst.name} has no update"
                    prev = (u0.ant_name, u0.id, cum[u0.ant_name])
                new_insts.append(inst)
            if changed:
                blk.instructions = new_insts


def build_program():
    nc = bacc.Bacc("TRN2", target_bir_lowering=False, debug=False)

    x_d = nc.dram_tensor("x", [B_PER_CORE, C, N], BF16, kind="ExternalInput").ap()
    wq_d = nc.dram_tensor("wqt", [C, C], BF16, kind="ExternalInput").ap()
    wk_d = nc.dram_tensor("wkt", [C, C], BF16, kind="ExternalInput").ap()
    wv_d = nc.dram_tensor("wvt", [C, C], BF16, kind="ExternalInput").ap()
    wo_d = nc.dram_tensor("wot", [C, C], BF16, kind="ExternalInput").ap()
    ebc_d = nc.dram_tensor("ebc", [2, 128], PROJ_DT, kind="ExternalInput").ap()
    out_d = nc.dram_tensor("out", [B_PER_CORE, C, N], BF16, kind="ExternalOutput").ap()

    with tile.TileContext(nc) as tc, ExitStack() as ctx:
        wp = ctx.enter_context(tc.tile_pool(name="w", bufs=1))
        xp = ctx.enter_context(tc.tile_pool(name="x", bufs=2))
        qkp = ctx.enter_context(tc.tile_pool(name="qk", bufs=1))
        vp = ctx.enter_context(tc.tile_pool(name="v", bufs=2))
        yp = ctx.enter_context(tc.tile_pool(name="y", bufs=2))
        smp = ctx.enter_context(tc.tile_pool(name="sm", bufs=2))
        abp = ctx.enter_context(tc.tile_pool(name="ab", bufs=2))
        otp = ctx.enter_context(tc.tile_pool(name="ot", bufs=8))
        ogp = ctx.enter_context(tc.tile_pool(name="og", bufs=3))
        cst = ctx.enter_context(tc.tile_pool(name="cst", bufs=1))
        drp = ctx.enter_context(tc.tile_pool(name="dr", bufs=2, space="DRAM"))

        ps_big = ctx.enter_context(tc.tile_pool(name="psb", bufs=5, space="PSUM"))
        ps_s4 = ctx.enter_context(tc.tile_pool(name="pss", bufs=1, space="PSUM"))
        ps_po = ctx.enter_context(tc.tile_pool(name="psp", bufs=2, space="PSUM"))

        eng4 = [nc.sync, nc.scalar, nc.gpsimd, nc.sync]

        # constants
        # ones2[e, hh]: column 0 sums the low 64 partitions, column 1 the high
        ones2 = cst.tile([128, 2], BF16)
        nc.vector.memset(ones2[:, :], 0.0)
        nc.vector.memset(ones2[0:64, 0:1], 1.0)
        nc.vector.memset(ones2[64:128, 1:2], 1.0)
        # E[hh, j]: broadcast recip row hh onto partition half hh (host const)
        e_bc = cst.tile([2, 128], PROJ_DT)
        nc.sync.dma_start(e_bc[:, :], ebc_d[:, :])

        w_sb = {}
        dengs = [nc.sync, nc.scalar, nc.gpsimd]
        _dq = [0]

        def _deng():
            e = dengs[_dq[0] % 3]
            _dq[0] += 1
            return e

        def alloc_w(name, dt=BF16):
            t = wp.tile([128, 4, C], dt, tag=name, name=f"w_{name}")
            w_sb[name] = t
            return t

        def load_w_half(name, d, h):
            dr = d.rearrange("(cc p) o -> p cc o", p=128)
            _deng().dma_start(
                w_sb[name][:, 2 * h : 2 * h + 2, :], dr[:, 2 * h : 2 * h + 2, :]
            )

        st = [{} for _ in range(B_PER_CORE)]

        X_SLICES = (slice(0, 128), slice(128, 512), slice(512, 1024))

        def s_load(b, sli=None):
            if sli is None or sli == 0:
                x_sb = xp.tile([128, 4, N], BF16, tag="xsb", name=f"x_sb{b}")
                st[b]["x"] = x_sb
            x_sb = st[b]["x"]
            xr = x_d[b].rearrange("(cc p) n -> p cc n", p=128)
            for s in range(3) if sli is None else [sli]:
                sl = X_SLICES[s]
                for cp in range(2):
                    _deng().dma_start(
                        x_sb[:, 2 * cp : 2 * cp + 2, sl],
                        xr[:, 2 * cp : 2 * cp + 2, sl],
                    )

        def s_proj_q(b):
            x_sb = st[b]["x"]
            atblk = abp.tile([128, 4, 128], BF16, tag="ab", name=f"ab{b}")
            nc.vector.memset(atblk[:, :, :], 0.0)
            st[b]["ab"] = atblk
            qt_sb = qkp.tile([128, 8, 512], BF16, tag="qt", name=f"qt{b}")
            st[b]["qt"] = qt_sb
            for ncn in range(8):
                nsl = slice(ncn * 128, (ncn + 1) * 128)
                pq = ps_big.tile([128, 512], F32, tag="big", name=f"pq{b}_{ncn}")
                if b == 0 and ncn < 2:
                    # first tiles: run in o-halves so the first matmul only
                    # waits for half of wq
                    for oh in range(2):
                        osl = slice(oh * 256, (oh + 1) * 256)
                        for cc in range(4):
                            nc.tensor.matmul(
                                pq[:, osl], x_sb[:, cc, nsl],
                                w_sb["wq"][:, cc, osl],
                                start=(cc == 0), stop=(cc == 3),
                            )
                else:
                    for cc in range(4):
                        nc.tensor.matmul(
                            pq[:, :], x_sb[:, cc, nsl], w_sb["wq"][:, cc, :],
                            start=(cc == 0), stop=(cc == 3),
                        )
                nc.vector.tensor_copy(qt_sb[:, ncn, :], pq[:, :])

        def s_proj_k_scores(b):
            x_sb, qt_sb = st[b]["x"], st[b]["qt"]
            kt_sb = qkp.tile([128, 8, 512], BF16, tag="kt", name=f"kt{b}")
            ps_s = ps_s4.tile([128, 4, 128], F32, tag="s4", name=f"ps_s{b}")
            st[b]["kt"], st[b]["ps_s"] = kt_sb, ps_s
            for ncn in range(8):
                nsl = slice(ncn * 128, (ncn + 1) * 128)
                pk = ps_big.tile([128, 512], F32, tag="big", name=f"pk{b}_{ncn}")
                for cc in range(4):
                    nc.tensor.matmul(
                        pk[:, :], x_sb[:, cc, nsl], w_sb["wk"][:, cc, :],
                        start=(cc == 0), stop=(cc == 3),
                    )
                nc.scalar.copy(kt_sb[:, ncn, :], pk[:, :])
            # scores: contiguous accumulation runs (PSUM accumulation groups
            # must not interleave on the tensor queue)
            for p in range(4):
                psl = slice(p * 128, (p + 1) * 128)
                for ncn in range(8):
                    nc.tensor.matmul(
                        ps_s[:, p, :],
                        kt_sb[:, ncn, psl],
                        qt_sb[:, ncn, psl],
                        start=(ncn == 0), stop=(ncn == 7),
                    )
            # exp of the diagonal 64x64 blocks (scoresT per head); one
            # activation per partition half covers all four head pairs
            et = smp.tile([128, 4, HD], BF16, tag="et", name=f"et{b}")
            for hh in range(2):
                s0 = hh * 64
                nc.scalar.activation(
                    et[s0 : s0 + 64, :, :],
                    ps_s[s0 : s0 + 64, :, s0 : s0 + 64],
                    AF.Exp, scale=0.125,
                )
            st[b]["et"] = et

        def s_proj_v(b, ocs=None):
            x_sb = st[b]["x"]
            if ocs is None or ocs[0] == 0:
                v_sb = vp.tile([128, 4, N], BF16, tag="vsb", name=f"v_sb{b}")
                st[b]["v"] = v_sb
            v_sb = st[b]["v"]
            for oc in (range(4) if ocs is None else ocs):
                for nh in range(2):
                    pv = ps_big.tile([128, 512], F32, tag="big", name=f"pv{b}_{oc}_{nh}")
                    for cc in range(4):
                        nc.tensor.matmul(
                            pv[:, :],
                            w_sb["wv"][:, cc, oc * 128 : (oc + 1) * 128],
                            x_sb[:, cc, nh * 512 : (nh + 1) * 512],
                            start=(cc == 0), stop=(cc == 3),
                        )
                    if nh == 0:
                        nc.vector.tensor_copy(v_sb[:, oc, 0:512], pv[:, :])
                    else:
                        nc.scalar.copy(v_sb[:, oc, 512:1024], pv[:, :])

        def s_softmax_aux(b):
            et, ps_s = st[b]["et"], st[b]["ps_s"]
            aux = ps_big.tile([128, 512], F32, tag="big", name=f"aux{b}")
            # batched column sums: aux[hh, p*64+d] = sum over partition half
            nc.tensor.matmul(
                aux[0:2, 0:256], ones2[:, :], et[:, :, :],
                start=True, stop=True,
            )
            recip2 = smp.tile([2, 256], PROJ_DT, tag="recip", name=f"recip{b}")
            with nc.allow_low_precision(reason="fp32r softmax denominators"):
                nc.vector.reciprocal(recip2[:, :], aux[0:2, 0:256])
            # broadcast recip row hh onto partition half hh: aux[:, 256:512]
            nc.tensor.matmul(
                aux[:, 256:512], e_bc[:, :], recip2[:, :],
                start=True, stop=True,
            )
            # block-diagonal normalized attn^T (bf16); one mul per half
            atblk = st[b]["ab"]
            auxv = aux[:, 256:512].rearrange("e (p d) -> e p d", p=4)
            for hh in range(2):
                s0 = hh * 64
                nc.vector.tensor_mul(
                    atblk[s0 : s0 + 64, :, s0 : s0 + 64],
                    et[s0 : s0 + 64, :, :],
                    auxv[s0 : s0 + 64, :, :],
                )
            st[b]["ab"] = atblk
            y_sb = yp.tile([128, 4, N], BF16, tag="ysb", name=f"y_sb{b}")
            st[b]["y"] = y_sb

        def s_outT(b):
            # block-diagonal attn@v: po [128 n-chunk, (h' d)] per (oc, ncn).
            # Scramble via DRAM bounce on async DMA rings (SBUF->SBUF static
            # DMAs execute as engine-blocking DIRECT2D; DRAM DMAs do not):
            #   fwd per (oc, half, h'): scr[oc, h', n, d] <- ot slices
            #   readback per oc: y_sb[:, oc, :] <- scr[oc] rearranged
            v_sb, atblk, y_sb = st[b]["v"], st[b]["ab"], st[b]["y"]
            scr = drp.tile([4, 2, N, HD], BF16, tag="scr", name=f"scr{b}")
            for oc in range(4):
                for half in range(2):
                    po = ps_po.tile(
                        [128, 4, 128], F32, tag="po", name=f"po{b}_{oc}_{half}"
                    )
                    for j in range(4):
                        ncn = half * 4 + j
                        nsl = slice(ncn * 128, (ncn + 1) * 128)
                        nc.tensor.matmul(
                            po[:, j, :], v_sb[:, oc, nsl], atblk[:, oc, :],
                            start=True, stop=True,
                        )
                    ot = otp.tile(
                        [128, 4, 128], BF16, tag="ot", name=f"ot{b}_{oc}_{half}"
                    )
                    if half == 0:
                        nc.vector.tensor_copy(ot[:, :, :], po[:, :, :])
                    else:
                        nc.scalar.copy(ot[:, :, :], po[:, :, :])
                    for hh in range(2):
                        dst = scr[
                            oc, hh, half * 512 : (half + 1) * 512, :
                        ].rearrange("(j nl) d -> nl j d", nl=128)
                        _deng().dma_start(dst, ot[:, :, hh * 64 : hh * 64 + 64])
                srcv = scr[oc].rearrange("h (a r) d -> h a (r d)", r=16)
                _deng().dma_start(y_sb[:, oc, :], srcv)

        def s_final(b):
            y_sb = st[b]["y"]
            for oc in range(4):
                og = ogp.tile([128, N], BF16, tag="og", name=f"og{b}_{oc}")
                for sh in range(2):
                    pf = ps_big.tile([128, 512], F32, tag="big", name=f"pf{b}_{oc}_{sh}")
                    for cp in range(4):
                        nc.tensor.matmul(
                            pf[:, :],
                            w_sb["wo"][:, cp, oc * 128 : (oc + 1) * 128],
                            y_sb[:, cp, sh * 512 : (sh + 1) * 512],
                            start=(cp == 0), stop=(cp == 3),
                        )
                    sl = slice(sh * 512, (sh + 1) * 512)
                    if sh == 0:
                        nc.vector.tensor_copy(og[:, sl], pf[:, :])
                    else:
                        nc.scalar.copy(og[:, sl], pf[:, :])
                    # spread output DMA across queues in 256-col halves
                    for q in range(2):
                        qsl = slice(sh * 512 + q * 256, sh * 512 + (q + 1) * 256)
                        _deng().dma_start(
                            out_d[b, oc * 128 : (oc + 1) * 128, qsl], og[:, qsl]
                        )

        # ---- schedule: two-batch software pipeline ----
        # initial loads in priority order, round-robin across DMA engines
        for nm in ("wq", "wk", "wv", "wo"):
            alloc_w(nm)
        wq_r = wq_d.rearrange("(cc p) o -> p cc o", p=128)
        for oh in range(2):
            for cc in range(4):
                _deng().dma_start(
                    w_sb["wq"][:, cc, oh * 256 : (oh + 1) * 256],
                    wq_r[:, cc, oh * 256 : (oh + 1) * 256],
                )
        s_load(0, sli=0)
        s_load(0, sli=1)
        s_load(0, sli=2)
        load_w_half("wk", wk_d, 0)
        load_w_half("wk", wk_d, 1)
        load_w_half("wv", wv_d, 0)
        load_w_half("wv", wv_d, 1)
        load_w_half("wo", wo_d, 0)
        load_w_half("wo", wo_d, 1)
        s_proj_q(0)
        s_load(1)
        s_proj_k_scores(0)
        s_proj_v(0, ocs=(0, 1, 2))
        s_softmax_aux(0)
        s_proj_v(0, ocs=(3,))
        s_outT(0)
        s_proj_q(1)
        s_proj_k_scores(1)
        s_proj_v(1, ocs=(0, 1, 2))
        s_softmax_aux(1)
        s_proj_v(1, ocs=(3,))
        s_final(0)
        s_outT(1)
        s_final(1)

    nc.compile()
    _split_excess_dma_waits(nc)
    return nc


_PROGRAM = None


def _get_program():
    global _PROGRAM
    if _PROGRAM is None:
        _PROGRAM = build_program()
    return _PROGRAM


def make_in_maps(x, Wq, Wk, Wv, Wo):
    import ml_dtypes
    x = np.ascontiguousarray(
        x.reshape(16, C, N).astype(ml_dtypes.bfloat16)
    )
    wqt = np.ascontiguousarray(Wq.T.astype(ml_dtypes.bfloat16))
    wkt = np.ascontiguousarray(Wk.T.astype(ml_dtypes.bfloat16))
    wvt = np.ascontiguousarray(Wv.T.astype(ml_dtypes.bfloat16))
    wot = np.ascontiguousarray(Wo.T.astype(ml_dtypes.bfloat16))
    ebc = np.zeros((2, 128), dtype=np.float32)
    ebc[0, 0:64] = 1.0
    ebc[1, 64:128] = 1.0
    in_maps = []
    for c in range(N_CORES):
        in_maps.append(
            {
                "x": np.ascontiguousarray(x[c * B_PER_CORE : (c + 1) * B_PER_CORE]),
                "wqt": wqt,
                "wkt": wkt,
                "wvt": wvt,
                "wot": wot,
                "ebc": ebc,
            }
        )
    return in_maps


def kernel(x, Wq, bq, Wk, bk, Wv, bv, Wo, bo, _trace=False):
    # biases are zeros by construction in this problem (spec fill="zeros");
    # they are not applied on-device.
    nc = _get_program()
    in_maps = make_in_maps(x, Wq, Wk, Wv, Wo)
    res = run_bass_kernel_spmd(nc, in_maps, list(range(N_CORES)), trace=_trace)
    outs = [
        np.asarray(res.results[c]["out"]).astype(np.float32)
        for c in range(N_CORES)
    ]
    full = np.concatenate(outs, axis=0).reshape(16, C, 32, 32)
    if _trace:
        return full, res
    return full


# revision 53
# speedup vs baseline: 1.0342x; 1.0342x over previous
"""Trainium2 Bass kernel for nn_MultiHeadAttentionBlock_49967649521921.

Reference computation (per batch b, x viewed as [C=512, N=1024]):
    q = Wq @ x ; k = Wk @ x ; v = Wv @ x          (1x1 convs, biases are zeros)
    per head h (8 heads, hd=64):
      scores[d,e] = sum_n q_h[d,n] k_h[e,n] / 8
      attn = softmax(scores, axis=e)
      out_h[d,n]  = sum_e attn[d,e] v_h[e,n]
    y[c',s'] = out[h, d, n] with c' = h*64 + n//16, s' = (n%16)*64 + d
    final = Wo @ y    -> reshape [512, 32, 32]

Sharding: data-parallel over batch. 16 batches / 8 cores = 2 per core.
No collectives; host scatters inputs and gathers outputs.

Device-side layouts (per core):
  x_sb  [128, 4, 1024]  channel-chunk-major view of x_b [C, N] (fp32/f32r)
  qt/kt [128, 8, 512]   bf16, spatial on partitions (q^T / k^T)
  v_sb  [128, 4, 1024]  bf16, [O, N] layout
  scoresT per head-PAIR in one [128,4,128] PSUM tile: matmul of the
  128-wide qt/kt window against itself gives both heads' scoresT in the
  two diagonal 64x64 blocks (bf16 runs 1 cycle/row at any width, so the
  discarded off-diagonal half is cheap). Softmax over partitions via a
  single batched ones-matmul column sum + reciprocal + one broadcast
  matmul. attn (normalized, transposed) lands in a BLOCK-DIAGONAL bf16
  tile atblk [128(e-pair), oc, 128(d-pair)] so attn@v for a head pair is
  ONE [128x128x128] bf16 matmul per 128-wide n-chunk: lhsT = v pair
  chunk (stationary), rhs = atblk -> po [128 n, 128 (d0|d1)].
  The reference's transpose(2,3).reshape scramble is realized as a DRAM
  bounce on async DMA rings (SBUF->SBUF static DMAs execute as
  engine-blocking DIRECT2D; DRAM DMAs do not): po halves are copied to
  ot tiles (bf16), forwarded per (oc, half, h') into scr[oc, h', n, d],
  and read back per oc as y_sb[:, oc, :] via a 3-dim rearrange.

Matmul dtypes: q/k/v/o projections run as float32r (full-rate PE at
moving>=256); the attention core runs bf16 (full rate at any width).
"""

import os
import sys

import numpy as np

for _p in ("/opt/trn_rl_repo",):
    if _p not in sys.path and os.path.isdir(_p):
        sys.path.insert(0, _p)

from contextlib import ExitStack

import concourse.bass as bass
import concourse.tile as tile
from concourse import bacc
from concourse import mybir
from concourse.bass_utils import run_bass_kernel_spmd

F32 = mybir.dt.float32
F32R = mybir.dt.float32r
BF16 = mybir.dt.bfloat16
AF = mybir.ActivationFunctionType

N_CORES = 8
B_PER_CORE = 2
C = 512
N = 1024
NH = 8
HD = 64

PROJ_DT = F32R   # q/k/v/o projections (moving free dim 512 -> full rate)


def _mm_cast(ap, dt):
    if ap.dtype == dt:
        return ap
    return ap.bitcast(dt)


def _split_excess_dma_waits(nc):
    """walrus' static-DMA (PSEUDO_DMA_DIRECT2D) encoding accepts a single
    sync-wait; Bacc's generate_event_semaphores only splits waits on compute
    instructions. Move excess DMA waits onto preceding EventSemaphore
    carriers (2 waits each) on the same engine queue."""
    for f in nc.m.functions:
        for blk in f.blocks:
            changed = False
            new_insts = []
            for inst in blk.instructions:
                si = inst.sync_info
                waits = list(si.on_wait) if si is not None and si.on_wait else []
                if inst.opcode in ("DMACopy", "DmaTransposeAnt") and len(waits) > 1:
                    keep, excess = waits[:1], waits[1:]
                    k = 0
                    while excess:
                        chunk, excess = excess[:2], excess[2:]
                        ev = mybir.InstEventSemaphore(
                            name=f"{inst.name}-evw{k}",
                            opcode="EventSemaphore",
                            engine=inst.engine,
                            sync_info=mybir.SyncInfo(on_wait=chunk, on_update=[]),
                        )
                        new_insts.append(ev)
                        k += 1
                    inst.sync_info = mybir.SyncInfo(
                        on_wait=keep, on_update=list(si.on_update or [])
                    )
                    changed = True
                new_insts.append(inst)
            if changed:
                blk.instructions = new_insts


def _serialize_transposes(nc):
    """DmaTransposeAnt fans out over all 16 DMA engines; two in-flight
    transposes interleave destructively. Chain them: each waits for the
    previous one's completion semaphore (cumulative +16 accounting, same
    program-order bookkeeping bacc uses for its own thresholds)."""
    from collections import defaultdict

    cum = defaultdict(int)
    prev = None
    for f in nc.m.functions:
        for blk in f.blocks:
            changed = False
            new_insts = []
            for inst in blk.instructions:
                si = inst.sync_info
                if si and si.on_update:
                    for u in si.on_update:
                        if u.ant_name and u.ant_name.startswith("DMA"):
                            cum[u.ant_name] += u.update_value
                if inst.opcode == "DmaTransposeAnt":
                    if prev is not None:
                        pname, pid, pcum = prev
                        ev = mybir.InstEventSemaphore(
                            name=f"{inst.name}-txchain",
                            opcode="EventSemaphore",
                            engine=inst.engine,
                            sync_info=mybir.SyncInfo(
                                on_wait=[
                                    mybir.SyncWait(
                                        sync_type="semaphore",
                                        id=pid,
                                        ant_name=pname,
                                        wait_mode="sem-ge-imm",
                                        wait_value=pcum,
                                    )
                                ],
                                on_update=[],
                            ),
                        )
                        new_insts.append(ev)
                        changed = True
                    u0 = (si.on_update or [None])[0]
                    assert u0 is not None, f"transpose {inst.name} has no update"
                    prev = (u0.ant_name, u0.id, cum[u0.ant_name])
                new_insts.append(inst)
            if changed:
                blk.instructions = new_insts


def build_program():
    nc = bacc.Bacc("TRN2", target_bir_lowering=False, debug=False)

    x_d = nc.dram_tensor("x", [B_PER_CORE, C, N], BF16, kind="ExternalInput").ap()
    wq_d = nc.dram_tensor("wqt", [C, C], BF16, kind="ExternalInput").ap()
    wk_d = nc.dram_tensor("wkt", [C, C], BF16, kind="ExternalInput").ap()
    wv_d = nc.dram_tensor("wvt", [C, C], BF16, kind="ExternalInput").ap()
    wo_d = nc.dram_tensor("wot", [C, C], BF16, kind="ExternalInput").ap()
    ebc_d = nc.dram_tensor("ebc", [2, 128], PROJ_DT, kind="ExternalInput").ap()
    out_d = nc.dram_tensor("out", [B_PER_CORE, C, N], BF16, kind="ExternalOutput").ap()

    with tile.TileContext(nc) as tc, ExitStack() as ctx:
        wp = ctx.enter_context(tc.tile_pool(name="w", bufs=1))
        xp = ctx.enter_context(tc.tile_pool(name="x", bufs=2))
        qkp = ctx.enter_context(tc.tile_pool(name="qk", bufs=1))
        vp = ctx.enter_context(tc.tile_pool(name="v", bufs=2))
        yp = ctx.enter_context(tc.tile_pool(name="y", bufs=2))
        smp = ctx.enter_context(tc.tile_pool(name="sm", bufs=2))
        abp = ctx.enter_context(tc.tile_pool(name="ab", bufs=2))
        otp = ctx.enter_context(tc.tile_pool(name="ot", bufs=8))
        ogp = ctx.enter_context(tc.tile_pool(name="og", bufs=3))
        cst = ctx.enter_context(tc.tile_pool(name="cst", bufs=1))
        drp = ctx.enter_context(tc.tile_pool(name="dr", bufs=2, space="DRAM"))

        ps_big = ctx.enter_context(tc.tile_pool(name="psb", bufs=5, space="PSUM"))
        ps_s4 = ctx.enter_context(tc.tile_pool(name="pss", bufs=1, space="PSUM"))
        ps_po = ctx.enter_context(tc.tile_pool(name="psp", bufs=2, space="PSUM"))

        eng4 = [nc.sync, nc.scalar, nc.gpsimd, nc.sync]

        # constants
        # ones2[e, hh]: column 0 sums the low 64 partitions, column 1 the high
        ones2 = cst.tile([128, 2], BF16)
        nc.vector.memset(ones2[:, :], 0.0)
        nc.vector.memset(ones2[0:64, 0:1], 1.0)
        nc.vector.memset(ones2[64:128, 1:2], 1.0)
        # E[hh, j]: broadcast recip row hh onto partition half hh (host const)
        e_bc = cst.tile([2, 128], PROJ_DT)
        nc.sync.dma_start(e_bc[:, :], ebc_d[:, :])

        w_sb = {}
        dengs = [nc.sync, nc.scalar, nc.gpsimd]
        _dq = [0]

        def _deng():
            e = dengs[_dq[0] % 3]
            _dq[0] += 1
            return e

        def alloc_w(name, dt=BF16):
            t = wp.tile([128, 4, C], dt, tag=name, name=f"w_{name}")
            w_sb[name] = t
            return t

        def load_w_half(name, d, h):
            dr = d.rearrange("(cc p) o -> p cc o", p=128)
            _deng().dma_start(
                w_sb[name][:, 2 * h : 2 * h + 2, :], dr[:, 2 * h : 2 * h + 2, :]
            )

        st = [{} for _ in range(B_PER_CORE)]

        X_SLICES = (slice(0, 128), slice(128, 512), slice(512, 1024))

        def s_load(b, sli=None):
            if sli is None or sli == 0:
                x_sb = xp.tile([128, 4, N], BF16, tag="xsb", name=f"x_sb{b}")
                st[b]["x"] = x_sb
            x_sb = st[b]["x"]
            xr = x_d[b].rearrange("(cc p) n -> p cc n", p=128)
            for s in range(3) if sli is None else [sli]:
                sl = X_SLICES[s]
                for cp in range(2):
                    _deng().dma_start(
                        x_sb[:, 2 * cp : 2 * cp + 2, sl],
                        xr[:, 2 * cp : 2 * cp + 2, sl],
                    )

        def s_proj_q(b):
            x_sb = st[b]["x"]
            atblk = abp.tile([128, 4, 128], BF16, tag="ab", name=f"ab{b}")
            nc.vector.memset(atblk[:, :, :], 0.0)
            st[b]["ab"] = atblk
            qt_sb = qkp.tile([128, 8, 512], BF16, tag="qt", name=f"qt{b}")
            st[b]["qt"] = qt_sb
            for ncn in range(8):
                nsl = slice(ncn * 128, (ncn + 1) * 128)
                pq = ps_big.tile([128, 512], F32, tag="big", name=f"pq{b}_{ncn}")
                if b == 0 and ncn < 2:
                    # first tiles: run in o-halves so the first matmul only
                    # waits for half of wq
                    for oh in range(2):
                        osl = slice(oh * 256, (oh + 1) * 256)
                        for cc in range(4):
                            nc.tensor.matmul(
                                pq[:, osl], x_sb[:, cc, nsl],
                                w_sb["wq"][:, cc, osl],
                                start=(cc == 0), stop=(cc == 3),
                            )
                else:
                    for cc in range(4):
                        nc.tensor.matmul(
                            pq[:, :], x_sb[:, cc, nsl], w_sb["wq"][:, cc, :],
                            start=(cc == 0), stop=(cc == 3),
                        )
                nc.vector.tensor_copy(qt_sb[:, ncn, :], pq[:, :])

        def s_proj_k_scores(b):
            x_sb, qt_sb = st[b]["x"], st[b]["qt"]
            kt_sb = qkp.tile([128, 8, 512], BF16, tag="kt", name=f"kt{b}")
            ps_s = ps_s4.tile([128, 4, 128], F32, tag="s4", name=f"ps_s{b}")
            st[b]["kt"], st[b]["ps_s"] = kt_sb, ps_s
            for ncn in range(8):
                nsl = slice(ncn * 128, (ncn + 1) * 128)
                pk = ps_big.tile([128, 512], F32, tag="big", name=f"pk{b}_{ncn}")
                for cc in range(4):
                    nc.tensor.matmul(
                        pk[:, :], x_sb[:, cc, nsl], w_sb["wk"][:, cc, :],
                        start=(cc == 0), stop=(cc == 3),
                    )
                nc.scalar.copy(kt_sb[:, ncn, :], pk[:, :])
            # scores: contiguous accumulation runs (PSUM accumulation groups
            # must not interleave on the tensor queue)
            for p in range(4):
                psl = slice(p * 128, (p + 1) * 128)
                for ncn in range(8):
                    nc.tensor.matmul(
                        ps_s[:, p, :],
                        kt_sb[:, ncn, psl],
                        qt_sb[:, ncn, psl],
                        start=(ncn == 0), stop=(ncn == 7),
                    )
            # exp of the diagonal 64x64 blocks (scoresT per head); one
            # activation per partition half covers all four head pairs
            et = smp.tile([128, 4, HD], BF16, tag="et", name=f"et{b}")
            for hh in range(2):
                s0 = hh * 64
                nc.scalar.activation(
                    et[s0 : s0 + 64, :, :],
                    ps_s[s0 : s0 + 64, :, s0 : s0 + 64],
                    AF.Exp, scale=0.125,
                )
            st[b]["et"] = et

        def s_proj_v(b, ocs=None):
            x_sb = st[b]["x"]
            if ocs is None or ocs[0] == 0:
                v_sb = vp.tile([128, 4, N], BF16, tag="vsb", name=f"v_sb{b}")
                st[b]["v"] = v_sb
            v_sb = st[b]["v"]
            for oc in (range(4) if ocs is None else ocs):
                for nh in range(2):
                    pv = ps_big.tile([128, 512], F32, tag="big", name=f"pv{b}_{oc}_{nh}")
                    for cc in range(4):
                        nc.tensor.matmul(
                            pv[:, :],
                            w_sb["wv"][:, cc, oc * 128 : (oc + 1) * 128],
                            x_sb[:, cc, nh * 512 : (nh + 1) * 512],
                            start=(cc == 0), stop=(cc == 3),
                        )
                    if nh == 0:
                        nc.vector.tensor_copy(v_sb[:, oc, 0:512], pv[:, :])
                    else:
                        nc.scalar.copy(v_sb[:, oc, 512:1024], pv[:, :])

        def s_softmax_aux(b):
            et, ps_s = st[b]["et"], st[b]["ps_s"]
            aux = ps_big.tile([128, 512], F32, tag="big", name=f"aux{b}")
            # batched column sums: aux[hh, p*64+d] = sum over partition half
            nc.tensor.matmul(
                aux[0:2, 0:256], ones2[:, :], et[:, :, :],
                start=True, stop=True,
            )
            recip2 = smp.tile([2, 256], PROJ_DT, tag="recip", name=f"recip{b}")
            with nc.allow_low_precision(reason="fp32r softmax denominators"):
                nc.vector.reciprocal(recip2[:, :], aux[0:2, 0:256])
            # broadcast recip row hh onto partition half hh: aux[:, 256:512]
            nc.tensor.matmul(
                aux[:, 256:512], e_bc[:, :], recip2[:, :],
                start=True, stop=True,
            )
            # block-diagonal normalized attn^T (bf16); one mul per half
            atblk = st[b]["ab"]
            auxv = aux[:, 256:512].rearrange("e (p d) -> e p d", p=4)
            for hh in range(2):
                s0 = hh * 64
                nc.vector.tensor_mul(
                    atblk[s0 : s0 + 64, :, s0 : s0 + 64],
                    et[s0 : s0 + 64, :, :],
                    auxv[s0 : s0 + 64, :, :],
                )
            st[b]["ab"] = atblk
            y_sb = yp.tile([128, 4, N], BF16, tag="ysb", name=f"y_sb{b}")
            st[b]["y"] = y_sb

        def s_outT(b):
            # block-diagonal attn@v: po [128 n-chunk, (h' d)] per (oc, ncn).
            # Scramble via DRAM bounce on async DMA rings (SBUF->SBUF static
            # DMAs execute as engine-blocking DIRECT2D; DRAM DMAs do not):
            #   fwd per (oc, half, h'): scr[oc, h', n, d] <- ot slices
            #   readback per oc: y_sb[:, oc, :] <- scr[oc] rearranged
            v_sb, atblk, y_sb = st[b]["v"], st[b]["ab"], st[b]["y"]
            scr = drp.tile([4, 2, N, HD], BF16, tag="scr", name=f"scr{b}")
            for oc in range(4):
                for half in range(2):
                    po = ps_po.tile(
                        [128, 4, 128], F32, tag="po", name=f"po{b}_{oc}_{half}"
                    )
                    for j in range(4):
                        ncn = half * 4 + j
                        nsl = slice(ncn * 128, (ncn + 1) * 128)
                        nc.tensor.matmul(
                            po[:, j, :], v_sb[:, oc, nsl], atblk[:, oc, :],
                            start=True, stop=True,
                        )
                    ot = otp.tile(
                        [128, 4, 128], BF16, tag="ot", name=f"ot{b}_{oc}_{half}"
                    )
                    if half == 0:
                        nc.vector.tensor_copy(ot[:, :, :], po[:, :, :])
                    else:
                        nc.scalar.copy(ot[:, :, :], po[:, :, :])
                    for hh in range(2):
                        dst = scr[
                            oc, hh, half * 512 : (half + 1) * 512, :
                        ].rearrange("(j nl) d -> nl j d", nl=128)
                        _deng().dma_start(dst, ot[:, :, hh * 64 : hh * 64 + 64])
                srcv = scr[oc].rearrange("h (a r) d -> h a (r d)", r=16)
                _deng().dma_start(y_sb[:, oc, :], srcv)

        def s_final(b):
            y_sb = st[b]["y"]
            for oc in range(4):
                og = ogp.tile([128, N], BF16, tag="og", name=f"og{b}_{oc}")
                for sh in range(2):
                    pf = ps_big.tile([128, 512], F32, tag="big", name=f"pf{b}_{oc}_{sh}")
                    for cp in range(4):
                        nc.tensor.matmul(
                            pf[:, :],
                            w_sb["wo"][:, cp, oc * 128 : (oc + 1) * 128],
                            y_sb[:, cp, sh * 512 : (sh + 1) * 512],
                            start=(cp == 0), stop=(cp == 3),
                        )
                    sl = slice(sh * 512, (sh + 1) * 512)
                    if sh == 0:
                        nc.vector.tensor_copy(og[:, sl], pf[:, :])
                    else:
                        nc.scalar.copy(og[:, sl], pf[:, :])
                    # spread output DMA across queues in 256-col halves
                    for q in range(2):
                        qsl = slice(sh * 512 + q * 256, sh * 512 + (q + 1) * 256)
                        _deng().dma_start(
                            out_d[b, oc * 128 : (oc + 1) * 128, qsl], og[:, qsl]
                        )

        # ---- schedule: two-batch software pipeline ----
        # initial loads in priority order, round-robin across DMA engines
        for nm in ("wq", "wk", "wv", "wo"):
            alloc_w(nm)
        wq_r = wq_d.rearrange("(cc p) o -> p cc o", p=128)
        for oh in range(2):
            for cc in range(4):
                _deng().dma_start(
                    w_sb["wq"][:, cc, oh * 256 : (oh + 1) * 256],
                    wq_r[:, cc, oh * 256 : (oh + 1) * 256],
                )
        s_load(0, sli=0)
        s_load(0, sli=1)
        s_load(0, sli=2)
        load_w_half("wk", wk_d, 0)
        load_w_half("wk", wk_d, 1)
        load_w_half("wv", wv_d, 0)
        load_w_half("wv", wv_d, 1)
        load_w_half("wo", wo_d, 0)
        load_w_half("wo", wo_d, 1)
        s_proj_q(0)
        s_load(1)
        s_proj_k_scores(0)
        s_proj_v(0, ocs=(0, 1, 2))
        s_softmax_aux(0)
        s_proj_v(0, ocs=(3,))
        s_outT(0)
        s_proj_q(1)
        s_proj_k_scores(1)
        s_proj_v(1, ocs=(0, 1, 2))
        s_softmax_aux(1)
        s_proj_v(1, ocs=(3,))
        s_outT(1)
        s_final(0)
        s_final(1)

    nc.compile()
    _split_excess_dma_waits(nc)
    return nc


_PROGRAM = None


def _get_program():
    global _PROGRAM
    if _PROGRAM is None:
        _PROGRAM = build_program()
    return _PROGRAM


def make_in_maps(x, Wq, Wk, Wv, Wo):
    import ml_dtypes
    x = np.ascontiguousarray(
        x.reshape(16, C, N).astype(ml_dtypes.bfloat16)
    )
    wqt = np.ascontiguousarray(Wq.T.astype(ml_dtypes.bfloat16))
    wkt = np.ascontiguousarray(Wk.T.astype(ml_dtypes.bfloat16))
    wvt = np.ascontiguousarray(Wv.T.astype(ml_dtypes.bfloat16))
    wot = np.ascontiguousarray(Wo.T.astype(ml_dtypes.bfloat16))
    ebc = np.zeros((2, 128), dtype=np.float32)
    ebc[0, 0:64] = 1.0
    ebc[1, 64:128] = 1.0
    in_maps = []
    for c in range(N_CORES):
        in_maps.append(
            {
                "x": np.ascontiguousarray(x[c * B_PER_CORE : (c + 1) * B_PER_CORE]),
                "wqt": wqt,
                "wkt": wkt,
                "wvt": wvt,
                "wot": wot,
                "ebc": ebc,
            }
        )
    return in_maps


def kernel(x, Wq, bq, Wk, bk, Wv, bv, Wo, bo, _trace=False):
    # biases are zeros by construction in this problem (spec fill="zeros");
    # they are not applied on-device.
    nc = _get_program()
    in_maps = make_in_maps(x, Wq, Wk, Wv, Wo)
    res = run_bass_kernel_spmd(nc, in_maps, list(range(N_CORES)), trace=_trace)
    outs = [
        np.asarray(res.results[c]["out"]).astype(np.float32)
        for c in range(N_CORES)
    ]
    full = np.concatenate(outs, axis=0).reshape(16, C, 32, 32)
    if _trace:
        return full, res
    return full
